# revision 1
# baseline (speedup 1.0000x reference)
"""KMeans assignment (vq_codebook) Trainium2 kernel.

argmin_k ||x_b - c_k||^2 for X[65536,1024], C[1024,1024], 8 NeuronCores,
data-parallel over the batch (8192 rows/core), centroids replicated.

Math: argmin_k d2 = argmax_k (X@C^T - ||c||^2/2); row term ||x||^2 dropped.
The cross term is computed to ~fp32 accuracy with 3 float32r matmuls via an
exact hi/lo mantissa split (fp22 truncation makes each product exact):
  X = Xh + Xl, C = Ch + Cl (hi = top 11 mantissa bits)
  X@C^T ~= Xh@Ch^T + Xh@Cl^T + Xl@Ch^T   (dropped Xl@Cl^T ~ 2^-24 rel)
The ||c||^2/2 bias is computed on device, broadcast to all partitions, and
subtracted on the Vector engine; argmax uses the DVE max/max_index ops.
"""
import numpy as np
import concourse.bacc as bacc
import concourse.mybir as mybir
from concourse.tile import TileContext
from concourse.bass_utils import run_bass_kernel_spmd

B, F, K = 65536, 1024, 1024
NCORES = 8
BL = B // NCORES          # rows per core
P = 128
FCH = F // P              # 8 contraction chunks
NH = 512                  # psum half (max fp32 moving operand / bank)
BBLK = 1024               # rows per X DMA block (2KB lines: full DMA bandwidth)
NBLK = BL // BBLK
TPB = BBLK // P           # b-tiles per block
DT = mybir.dt.bfloat16

_NC_CACHE = {}


def _build(bl):
    nblk = bl // BBLK
    nb = bl // P
    nc = bacc.Bacc("TRN2", target_bir_lowering=False)
    xh = nc.dram_tensor("xh", [F, bl], DT, kind="ExternalInput")
    xl = nc.dram_tensor("xl", [F, bl], DT, kind="ExternalInput")
    ch = nc.dram_tensor("ch", [F, K], DT, kind="ExternalInput")
    cl = nc.dram_tensor("cl", [F, K], DT, kind="ExternalInput")
    cc = nc.dram_tensor("cc", [K, F], mybir.dt.float32, kind="ExternalInput")
    out = nc.dram_tensor("out", [nb, P, 1], mybir.dt.uint32, kind="ExternalOutput")
    c2lin = nc.dram_tensor("c2lin", [K], mybir.dt.float32, kind="Internal")

    xh_r = xh.rearrange("(fo p) b -> p fo b", p=P)
    xl_r = xl.rearrange("(fo p) b -> p fo b", p=P)

    with TileContext(nc) as tc:
        with (
            tc.tile_pool(name="cres", bufs=1) as cres,
            tc.tile_pool(name="xp", bufs=2) as xp,
            tc.tile_pool(name="work", bufs=3) as work,
            tc.tile_pool(name="psp", bufs=4, space="PSUM") as psp,
        ):
            # resident transposed centroid tiles (hi/lo split); one tile per
            # f-chunk so the first matmul only waits on chunk 0's DMA.
            # Issue order: C chunks + block-0 X chunks first (PE-critical),
            # cc + the c2 chain after (only needed by the first DVE sub,
            # which PSUM bufs=4 pushes ~40us out).
            def load_blk_chunk(blk, f):
                b0 = blk * BBLK
                t_h = xp.tile([P, BBLK], DT, tag=f"xh{f}")
                t_l = xp.tile([P, BBLK], DT, tag=f"xl{f}")
                nc.sync.dma_start(t_h, xh[f * P:(f + 1) * P, b0:b0 + BBLK])
                nc.sync.dma_start(t_l, xl[f * P:(f + 1) * P, b0:b0 + BBLK])
                return t_h, t_l

            def load_blk(blk):
                hs, ls = [], []
                for f in range(FCH):
                    t_h, t_l = load_blk_chunk(blk, f)
                    hs.append(t_h)
                    ls.append(t_l)
                return hs, ls

            # C chunks first (PE-critical, resident for the whole kernel),
            # then block-0's X chunks. Per-chunk tiles keep the first
            # matmuls gated only on the chunks they actually read.
            ch_sb = []
            cl_sb = []
            for f in range(FCH):
                t_h = cres.tile([P, K], DT, tag=f"ch{f}")
                t_l = cres.tile([P, K], DT, tag=f"cl{f}")
                nc.sync.dma_start(t_h, ch[f * P:(f + 1) * P, :])
                nc.sync.dma_start(t_l, cl[f * P:(f + 1) * P, :])
                ch_sb.append(t_h)
                cl_sb.append(t_l)

            blk0_tiles = load_blk(0)

            # c2/2 on device from row-major centroids (segmented reduce for
            # better fp32 accuracy), then scatter->broadcast via DRAM.
            c2pm = cres.tile([P, FCH], mybir.dt.float32)
            for j in range(FCH):
                cc_sb = work.tile([P, F], mybir.dt.float32, tag="ccsb")
                nc.sync.dma_start(cc_sb, cc[j * P:(j + 1) * P, :])
                csq = work.tile([P, F], mybir.dt.float32, tag="csq")
                nc.vector.tensor_mul(csq, cc_sb, cc_sb)
                seg = work.tile([P, 16], mybir.dt.float32, tag="seg")
                nc.vector.tensor_reduce(
                    seg, csq.rearrange("p (s t) -> p s t", t=64),
                    axis=mybir.AxisListType.X, op=mybir.AluOpType.add)
                nc.vector.tensor_reduce(
                    c2pm[:, j:j + 1], seg,
                    axis=mybir.AxisListType.X, op=mybir.AluOpType.add)
            nc.vector.tensor_scalar_mul(c2pm, c2pm, 0.5)
            nc.sync.dma_start(c2lin.rearrange("(j p) -> p j", p=P), c2pm)
            c2b = cres.tile([P, K], mybir.dt.float32)
            nc.sync.dma_start(c2b, c2lin[None, :].to_broadcast([P, K]))

            for blk in range(nblk):
                xh_t, xl_t = blk0_tiles if blk == 0 else load_blk(blk)
                for i in range(TPB):
                    t = blk * TPB + i
                    ps = psp.tile([P, K], mybir.dt.float32)
                    for f in range(FCH):
                        first = f == 0
                        last = f == FCH - 1
                        wh = xh_t[f][:, i * P:(i + 1) * P]
                        wl = xl_t[f][:, i * P:(i + 1) * P]
                        nc.tensor.matmul(ps[:, 0:NH], wh, ch_sb[f][:, 0:NH],
                                         start=first, stop=False)
                        nc.tensor.matmul(ps[:, NH:K], wh, ch_sb[f][:, NH:K],
                                         start=first, stop=False)
                        nc.tensor.matmul(ps[:, 0:NH], wh, cl_sb[f][:, 0:NH],
                                         start=False, stop=False)
                        nc.tensor.matmul(ps[:, NH:K], wh, cl_sb[f][:, NH:K],
                                         start=False, stop=False)
                        nc.tensor.matmul(ps[:, 0:NH], wl, ch_sb[f][:, 0:NH],
                                         start=False, stop=last)
                        nc.tensor.matmul(ps[:, NH:K], wl, ch_sb[f][:, NH:K],
                                         start=False, stop=last)
                    a_sb = work.tile([P, K], mybir.dt.float32, tag="a")
                    nc.vector.tensor_sub(a_sb, ps, c2b)
                    mx = work.tile([P, 8], mybir.dt.float32, tag="mx")
                    nc.vector.max(out=mx, in_=a_sb)
                    ix = work.tile([P, 8], mybir.dt.uint32, tag="ix")
                    nc.vector.max_index(ix, mx, a_sb)
                    nc.sync.dma_start(out[t], ix[:, 0:1])
    nc.finalize()
    return nc


def _split_hi_lo(a):
    """Split fp32 into two bf16 terms: a ~= hi + lo with ~2^-17 rel residue."""
    import ml_dtypes
    hi = a.astype(ml_dtypes.bfloat16)
    lo = (a - hi.astype(np.float32)).astype(ml_dtypes.bfloat16)
    return hi, lo


def _get_nc(bl):
    if bl not in _NC_CACHE:
        _NC_CACHE[bl] = _build(bl)
    return _NC_CACHE[bl]


def kernel(X, centroids):
    X = np.ascontiguousarray(np.asarray(X, dtype=np.float32))
    C = np.ascontiguousarray(np.asarray(centroids, dtype=np.float32))
    assert X.shape == (B, F) and C.shape == (K, F)

    xt = np.ascontiguousarray(X.T)
    ct = np.ascontiguousarray(C.T)
    xh_all, xl_all = _split_hi_lo(xt)
    ch_t, cl_t = _split_hi_lo(ct)

    nc = _get_nc(BL)
    in_maps = []
    for c in range(NCORES):
        sl = slice(c * BL, (c + 1) * BL)
        in_maps.append({
            "xh": np.ascontiguousarray(xh_all[:, sl]),
            "xl": np.ascontiguousarray(xl_all[:, sl]),
            "ch": ch_t,
            "cl": cl_t,
            "cc": C,
        })
    res = run_bass_kernel_spmd(nc, in_maps, core_ids=list(range(NCORES)))
    out = np.concatenate([r["out"].reshape(-1) for r in res.results])
    return out.astype(np.int32)



# revision 4
# speedup vs baseline: 1.9343x; 1.9343x over previous
"""KMeans assignment (vq_codebook) Trainium2 kernel.

argmin_k ||x_b - c_k||^2 for X[65536,1024], C[1024,1024], 8 NeuronCores,
data-parallel over the batch (8192 rows/core), centroids replicated.

Math: argmin_k d2 = argmax_k (X@C^T - ||c||^2/2); row term ||x||^2 dropped.
The cross term is computed as a scaled fp16 main matmul plus one fused
fp8e4m3 DoubleRow correction matmul (2x PE throughput) that adds both
first-order residue terms in the same PSUM accumulation group:

  X = Xh + Xl  (Xh = fp16 round, applied to 2^12*X so the main matmul
                already carries the 2^12 scale exactly)
  C = Ch + Cl  (Ch = fp16 round)
  psum = (2^12*Xh)@Ch                                   [fp16 matmul, exact]
       + fp8(2^12*Xl)@fp8(Ch) + fp8(Xh)@fp8(2^12*Cl)   [one DoubleRow matmul]
       ~= 2^12 * X@C   (dropped Xl@Cl ~ 2^-22 rel; fp8 quantization of the
                        correction operands leaves ~2e-4 abs score error)

The 2^12*||c||^2/2 bias is precomputed on host, broadcast to all partitions,
subtracted on the Vector engine; argmax uses the DVE max/max_index ops.
"""
import numpy as np
import ml_dtypes
import concourse.bacc as bacc
import concourse.mybir as mybir
from concourse.tile import TileContext
from concourse.bass_utils import run_bass_kernel_spmd

B, F, K = 65536, 1024, 1024
NCORES = 8
BL = B // NCORES          # rows per core
P = 128
FCH = F // P              # 8 contraction chunks
NH = 512                  # psum half (one bank of fp32)
BBLK = 1024               # rows per X DMA block (2KB+ lines: full DMA bw)
NBLK = BL // BBLK
TPB = BBLK // P           # b-tiles per block
S = 4096.0                # 2^12 scale carried by the X side / psum / bias
DT16 = mybir.dt.float16
DT8 = mybir.dt.float8e4

_NC_CACHE = {}


def _build(bl):
    nblk = bl // BBLK
    nb = bl // P
    nc = bacc.Bacc("TRN2", target_bir_lowering=False)
    xh = nc.dram_tensor("xh", [F, bl], DT16, kind="ExternalInput")
    xp = nc.dram_tensor("xp", [F, 2, bl], DT8, kind="ExternalInput")
    ch = nc.dram_tensor("ch", [F, K], DT16, kind="ExternalInput")
    cp = nc.dram_tensor("cp", [F, 2, K], DT8, kind="ExternalInput")
    c2s = nc.dram_tensor("c2s", [K], mybir.dt.float32, kind="ExternalInput")
    out = nc.dram_tensor("out", [nb, P, 1], mybir.dt.uint32, kind="ExternalOutput")

    with TileContext(nc) as tc:
        with (
            tc.tile_pool(name="cres", bufs=1) as cres,
            tc.tile_pool(name="xpool", bufs=2) as xpool,
            tc.tile_pool(name="work", bufs=3) as work,
            tc.tile_pool(name="psp", bufs=4, space="PSUM") as psp,
        ):
            def load_blk(blk):
                b0 = blk * BBLK
                hs, ps8 = [], []
                for f in range(FCH):
                    t_h = xpool.tile([P, BBLK], DT16, tag=f"xh{f}")
                    t_p = xpool.tile([P, 2, BBLK], DT8, tag=f"xp{f}")
                    nc.sync.dma_start(t_h, xh[f * P:(f + 1) * P, b0:b0 + BBLK])
                    nc.sync.dma_start(t_p, xp[f * P:(f + 1) * P, :, b0:b0 + BBLK])
                    hs.append(t_h)
                    ps8.append(t_p)
                return hs, ps8

            # C chunks first (PE-critical, resident for the whole kernel),
            # then block-0's X chunks. Per-chunk tiles keep the first
            # matmuls gated only on the chunks they actually read.
            ch_sb = []
            cp_sb = []
            for f in range(FCH):
                t_h = cres.tile([P, K], DT16, tag=f"ch{f}")
                t_p = cres.tile([P, 2, K], DT8, tag=f"cp{f}")
                nc.sync.dma_start(t_h, ch[f * P:(f + 1) * P, :])
                nc.sync.dma_start(t_p, cp[f * P:(f + 1) * P, :, :])
                ch_sb.append(t_h)
                cp_sb.append(t_p)

            blk0_tiles = load_blk(0)

            # bias broadcast: c2s already holds 2^12 * ||c||^2 / 2
            c2b = cres.tile([P, K], mybir.dt.float32)
            nc.sync.dma_start(c2b, c2s[None, :].to_broadcast([P, K]))

            for blk in range(nblk):
                xh_t, xp_t = blk0_tiles if blk == 0 else load_blk(blk)
                for i in range(TPB):
                    t = blk * TPB + i
                    ps = psp.tile([P, K], mybir.dt.float32)
                    for f in range(FCH):
                        first = f == 0
                        last = f == FCH - 1
                        wh = xh_t[f][:, i * P:(i + 1) * P]
                        wp = xp_t[f][:, :, i * P:(i + 1) * P]
                        nc.tensor.matmul(ps[:, 0:NH], wh, ch_sb[f][:, 0:NH],
                                         start=first, stop=False)
                        nc.tensor.matmul(ps[:, NH:K], wh, ch_sb[f][:, NH:K],
                                         start=first, stop=False)
                        nc.tensor.matmul(ps[:, 0:NH], wp, cp_sb[f][:, :, 0:NH],
                                         start=False, stop=last,
                                         perf_mode=mybir.MatmulPerfMode.DoubleRow)
                        nc.tensor.matmul(ps[:, NH:K], wp, cp_sb[f][:, :, NH:K],
                                         start=False, stop=last,
                                         perf_mode=mybir.MatmulPerfMode.DoubleRow)
                    a_sb = work.tile([P, K], mybir.dt.float32, tag="a")
                    nc.vector.tensor_sub(a_sb, ps, c2b)
                    mx = work.tile([P, 8], mybir.dt.float32, tag="mx")
                    nc.vector.max(out=mx, in_=a_sb)
                    ix = work.tile([P, 8], mybir.dt.uint32, tag="ix")
                    nc.vector.max_index(ix, mx, a_sb)
                    nc.sync.dma_start(out[t], ix[:, 0:1])
    nc.finalize()
    return nc


def _get_nc(bl):
    if bl not in _NC_CACHE:
        _NC_CACHE[bl] = _build(bl)
    return _NC_CACHE[bl]


def _prep(X, C):
    """Host-side operand prep (fp16 main + fp8 correction pack)."""
    f8 = ml_dtypes.float8_e4m3

    xs = (X * np.float32(S)).astype(np.float32)      # 2^12 * X
    xh16 = xs.astype(np.float16)                     # fp16 main operand (scaled)
    xh32 = xh16.astype(np.float32)
    xl = xs - xh32                                   # 2^12 * Xl, exact in fp32
    ch16 = C.astype(np.float16)
    ch32 = ch16.astype(np.float32)
    cl = (C - ch32) * np.float32(S)                  # 2^12 * Cl

    xp = np.empty((F, 2, B), dtype=f8)
    xp[:, 0, :] = xl.T.astype(f8)                    # fp8(2^12*Xl)
    xp[:, 1, :] = (xh32.T * np.float32(1.0 / S)).astype(f8)   # fp8(Xh)
    cp = np.empty((F, 2, K), dtype=f8)
    cp[:, 0, :] = ch32.T.astype(f8)                  # fp8(Ch)
    cp[:, 1, :] = cl.T.astype(f8)                    # fp8(2^12*Cl)

    xh_t = np.ascontiguousarray(xh16.T)
    ch_t = np.ascontiguousarray(ch16.T)
    c2s = (np.float64(S) * 0.5 * np.sum(C.astype(np.float64) ** 2, axis=1)
           ).astype(np.float32)
    return xh_t, xp, ch_t, cp, c2s


def kernel(X, centroids):
    X = np.ascontiguousarray(np.asarray(X, dtype=np.float32))
    C = np.ascontiguousarray(np.asarray(centroids, dtype=np.float32))
    assert X.shape == (B, F) and C.shape == (K, F)

    xh_t, xp, ch_t, cp, c2s = _prep(X, C)

    nc = _get_nc(BL)
    in_maps = []
    for c in range(NCORES):
        sl = slice(c * BL, (c + 1) * BL)
        in_maps.append({
            "xh": np.ascontiguousarray(xh_t[:, sl]),
            "xp": np.ascontiguousarray(xp[:, :, sl]),
            "ch": ch_t,
            "cp": cp,
            "c2s": c2s,
        })
    res = run_bass_kernel_spmd(nc, in_maps, core_ids=list(range(NCORES)))
    out = np.concatenate([r["out"].reshape(-1) for r in res.results])
    return out.astype(np.int32)
